# revision 17
# baseline (speedup 1.0000x reference)
"""Fused per-pixel kernel for nn_KernelFusion_19026705121450 on 8 trn2 cores.

Math: per pixel q = z[b,:,h,w] (3 ch), per batch t = Wt text + bt:
    z_map = Wz q + bz; dist = ||z_map - t||^2; kl = z_map . t
    k = (w0 e^{-g dist} + w1 kl + w2 (a kl + c)^2) / (sum w + 1e-8)
    out = Wo (z_map (1 + sigmoid(k))) + bo

All 64-dim reductions collapse (host, fp64) to 3-dim forms:
    dist = ||L^T q + r||^2 + rho   (L = chol(Wz^T Wz))
    kl   = u . q + s
    out_o = (M_o . q + m_o) g + bo_o,  M = Wo Wz, g = 1.5 + 0.5 tanh(k/2)

Device: one 1024-col pass over [128, 1024] fp16 tiles (partition =
batch*64 + rowblock, free = pixel). Forms are pivot-normalized on their
lead channel so biases ride tensor_scalar const slots; pivot scales
refold into ACT Square scales / per-o g1 consts. tanh (same ACT table
as exp/square) replaces sigmoid to avoid a table reload; a warmup ACT
op preloads the table before DMAs land. MACs decompose per cfg across
DVE (ts+tt / stt / ln_bwd custom op) and Pool (ts half).
"""

import sys

if "/opt/trn_rl_repo" not in sys.path:
    sys.path.insert(0, "/opt/trn_rl_repo")

import numpy as np

import concourse.bass as bass
import concourse.bacc as bacc
import concourse.mybir as mybir
from concourse.tile import TileContext
from concourse import bass_utils

F32 = mybir.dt.float32
F16 = mybir.dt.float16
AF = mybir.ActivationFunctionType
OP = mybir.AluOpType

NCORES = 8
BPC = 2          # batches per core
ROWS = 64        # partition rows per batch
P = 128
FREE = 1024

# const column indices (fp32 tensor)
# form f: z_lead + a1*z_a + a2*z_b + bias  (negated copies for ln mode)
C_A1E0, C_BE0, C_A2E0, C_SQ0S = 0, 1, 2, 3
C_A1E1, C_BE1, C_SQ1S = 4, 5, 6
C_SQ2S, C_SQ2B = 7, 8
C_NEGG, C_BETA0 = 9, 10
C_A1KL, C_BKL, C_A2KL = 11, 12, 13
C_P2S, C_P2B, C_W1U0 = 14, 15, 16
C_A1Y, C_BY, C_A2Y = 17, 20, 23       # +o
C_G1S, C_G1B = 26, 29                 # +o
C_BO = 32                             # +o
C_NA1E0, C_NBE0 = 35, 36              # negated (for ln_bwd mode)
C_NA1E1, C_NBE1 = 37, 38
C_NA1KL, C_NBKL = 39, 40
C_NA1Y, C_NBY = 41, 44                # +o
NCONST = 47

_NC_CACHE: dict = {}

# Best found schedule: dist path first (krbf fires early), y-form work
# fills DVE afterward, y0 muls ride ACT idle slots, outputs on the SP
# hardware-DGE queue. Measured 20661 ns in TimelineSim (8-core SPMD).
ORDER_BEST = (
    "klm1", "e0m1", "e1m",
    "e0m2", "klm2", "y1m1", "y2m1", "y1m2", "y2m2",
    "sq2",
    "e0a1", "kla1", "e1a",
    "sq1",
    "e0a2", "kla2",
    "sq0",
    "tpm", "d1", "d2",
    "p2", "krbf",
    "tpa", "t2",
    "th",
    "y0m1", "y0m2", "y0a1", "y1a1",
    "g11", "g12",
    "y0a2", "y1a2", "y2a1", "y2a2",
    "g10", "v0", "v1", "v2",
)

BEST_CFG: dict = {"in_eng": ("act", "sync", "sync", "sync"),
                  "zorder": "z2split", "act_ops": ("y0m1", "y0m2"),
                  "out_eng": ("sync", "sync", "sync"),
                  "dve_order": ORDER_BEST}


def _build_nc(sw0_pos: bool, sw2_pos: bool, bo_zero: bool, cfg: dict | None):
    """Emission order is hand-scheduled for the in-order engines.

    Step names (used by the `plan` cfg: list of (step, engine) pairs, where
    engine is 'dve'|'pool'|'act' for compute placement where it matters):
      mul ops ("<form>m1" = ts of z_aux w/ bias, "<form>m2" = ts of z2),
      adds ("<form>a1" lead+m1, "<form>a2" +m2), squares/exp/tanh on ACT
      fixed, d1/d2/t2 adds, tpoly stt, g1_o, v_o.
    """
    cfg = dict(cfg or {})
    warm = cfg.get("warm", True)
    # consts, z1, z0, z2 DMA queues
    in_eng = cfg.get("in_eng", ("act", "sync", "sync", "pool"))
    out_eng = cfg.get("out_eng", ("sync", "pool", "act"))
    # placement of the movable mul/aux ops
    pool_ops = set(cfg.get("pool_ops",
                           ("e0m2", "klm2", "y1m1", "y2m1", "y1m2",
                            "y2m2")))
    act_ops = set(cfg.get("act_ops", ("e1m1", "y0m2")))
    dve_order = cfg.get("dve_order", None)
    g1_act = set(cfg.get("g1_act", (1, 2)))     # g1 indices on ACT

    nc = bacc.Bacc("TRN2", target_bir_lowering=False)
    cons = nc.dram_tensor("consts", [P, NCONST], F32, kind="ExternalInput")
    z01 = nc.dram_tensor("z01", [P, 2 * FREE], F16, kind="ExternalInput")
    z2d = nc.dram_tensor("z2", [P, FREE], F16, kind="ExternalInput")
    outs = [nc.dram_tensor(f"o{o}", [P, FREE], F16, kind="ExternalOutput")
            for o in range(3)]

    op_w2 = OP.add if sw2_pos else OP.subtract

    def dmaeng(which):
        return {"sync": nc.sync, "pool": nc.gpsimd, "act": nc.scalar,
                "dve": nc.vector}[which]

    with TileContext(nc) as tc:
        with tc.tile_pool(name="cpool", bufs=1) as cpool, \
             tc.tile_pool(name="work", bufs=1) as pool:
            ct = cpool.tile([P, NCONST], F32, name="ct")
            zt = cpool.tile([P, 2 * FREE], F16, name="zt")
            z2t = cpool.tile([P, FREE], F16, name="z2t")
            wt = cpool.tile([P, 1], F32, name="wt")

            if warm:
                nc.vector.memset(wt[:, :], 0.0)
                nc.scalar.activation(wt[:, :], wt[:, :], AF.Square)

            # packed z01 = [z1 | z0]
            zorder = cfg.get("zorder", "z2first")
            dmaeng(in_eng[0]).dma_start(out=ct[:, :], in_=cons[:, :])
            if zorder == "z2first":
                dmaeng(in_eng[3]).dma_start(out=z2t[:, :], in_=z2d[:, :])
                dmaeng(in_eng[1]).dma_start(out=zt[:, :], in_=z01[:, :])
            elif zorder == "z01first":
                dmaeng(in_eng[1]).dma_start(out=zt[:, :], in_=z01[:, :])
                dmaeng(in_eng[3]).dma_start(out=z2t[:, :], in_=z2d[:, :])
            elif zorder == "split":
                dmaeng(in_eng[1]).dma_start(out=zt[:, 0:FREE],
                                            in_=z01[:, 0:FREE])
                dmaeng(in_eng[3]).dma_start(out=z2t[:, :], in_=z2d[:, :])
                dmaeng(in_eng[2]).dma_start(out=zt[:, FREE:2 * FREE],
                                            in_=z01[:, FREE:2 * FREE])
            elif zorder == "z2split":  # z2, z1, z0 (all split)
                dmaeng(in_eng[3]).dma_start(out=z2t[:, :], in_=z2d[:, :])
                dmaeng(in_eng[1]).dma_start(out=zt[:, 0:FREE],
                                            in_=z01[:, 0:FREE])
                dmaeng(in_eng[2]).dma_start(out=zt[:, FREE:2 * FREE],
                                            in_=z01[:, FREE:2 * FREE])
            else:  # z1first: z1, z2, z0
                dmaeng(in_eng[1]).dma_start(out=zt[:, 0:FREE],
                                            in_=z01[:, 0:FREE])
                dmaeng(in_eng[3]).dma_start(out=z2t[:, :], in_=z2d[:, :])
                dmaeng(in_eng[2]).dma_start(out=zt[:, FREE:2 * FREE],
                                            in_=z01[:, FREE:2 * FREE])
            z1 = zt[:, 0:FREE]
            z0 = zt[:, FREE:2 * FREE]
            z2 = z2t[:, :]

            def col(j):
                return ct[:, j:j + 1]

            tiles = {}

            def t(tag, w=FREE):
                if tag not in tiles:
                    tiles[tag] = pool.tile([P, w], F16, tag=tag, name=tag)
                return tiles[tag]

            def E(tag):
                return nc.gpsimd if tag in pool_ops else nc.vector

            def ts_op(tag, dst, src, scol, bcol):
                # dst = scol*src + bcol on pool/act/dve per placement
                if tag in act_ops:
                    nc.scalar.activation(dst, src, AF.Identity,
                                         bias=bcol if not isinstance(bcol, float)
                                         else bcol, scale=scol)
                else:
                    E(tag).tensor_scalar(dst, src, scol, bcol,
                                         OP.mult, OP.add)

            # ---------- op emitters (callable in any order) ----------
            # forms: f in {e0, e1, kl, y0, y1, y2}
            # e1 is 2-term: lead z1, aux z2. others: lead z0, aux z1 + z2.
            FORM = {
                "e0": (None, C_A1E0, C_BE0, C_A2E0, C_NA1E0, C_NBE0),
                "kl": (None, C_A1KL, C_BKL, C_A2KL, C_NA1KL, C_NBKL),
                "y0": (None, C_A1Y + 0, C_BY + 0, C_A2Y + 0, C_NA1Y, C_NBY),
                "y1": (None, C_A1Y + 1, C_BY + 1, C_A2Y + 1, C_NA1Y + 1,
                       C_NBY + 1),
                "y2": (None, C_A1Y + 2, C_BY + 2, C_A2Y + 2, C_NA1Y + 2,
                       C_NBY + 2),
            }

            ybatch = cfg.get("ybatch", False)

            def yslice(base, f):
                o = int(f[1])
                return t(base, 3 * FREE)[:, o * FREE:(o + 1) * FREE]

            def m1(f):      # tmp = a1*z1 + b   (aux mul with bias)
                _, a1, b, _, _, _ = FORM[f]
                if ybatch and f.startswith("y"):
                    dst = yslice("ymcat", f)
                else:
                    dst = t(f + "_m1")[:, :]
                ts_op(f + "m1", dst, z1, col(a1), col(b))

            def a1(f):      # acc = z0 + tmp
                nc.vector.tensor_add(out=t(f + "_a")[:, :], in0=z0,
                                     in1=t(f + "_m1")[:, :])

            def ln1(f):     # acc = z0 + a1*z1 + b via custom op (1 DVE op)
                _, _, _, _, na1, nb = FORM[f]
                nc.vector.ln_bwd_dx(t(f + "_a")[:, :], z0, z1, col(na1),
                                    col(nb))

            def m2(f):      # tmp2 = a2*z2
                _, _, _, a2, _, _ = FORM[f]
                if ybatch and f.startswith("y"):
                    dst = yslice("ym2cat", f)
                else:
                    dst = t(f + "_m2")[:, :]
                ts_op(f + "m2", dst, z2, col(a2), 0.0)

            def ya1b(_=None):   # all three y a1 joins in one wide op
                mc = t("ymcat", 3 * FREE)[:, :].rearrange(
                    "p (o f) -> p o f", o=3)
                ac = t("yacat", 3 * FREE)[:, :].rearrange(
                    "p (o f) -> p o f", o=3)
                zb = z0.unsqueeze(1).broadcast_to([P, 3, FREE])
                nc.vector.tensor_add(out=ac, in0=zb, in1=mc)

            def ya2b(_=None):   # all three y a2 joins in one wide op
                nc.vector.tensor_add(out=t("ycat", 3 * FREE)[:, :],
                                     in0=t("yacat", 3 * FREE)[:, :],
                                     in1=t("ym2cat", 3 * FREE)[:, :])

            def a2(f):      # out = acc + tmp2
                E(f + "a2").tensor_add(out=t(f)[:, :], in0=t(f + "_a")[:, :],
                                       in1=t(f + "_m2")[:, :])

            def s2(f):      # out = acc + a2*z2 via stt (skip m2)
                _, _, _, a2c, _, _ = FORM[f]
                nc.vector.scalar_tensor_tensor(t(f)[:, :], z2, col(a2c),
                                               t(f + "_a")[:, :], OP.mult,
                                               OP.add)

            def e1m(_=None):   # e1 aux: tmp = a1*z2 + b
                ts_op("e1m1", t("e1_m1")[:, :], z2, col(C_A1E1),
                      col(C_BE1))

            def e1a(_=None):   # e1 = z1 + tmp
                nc.vector.tensor_add(out=t("e1")[:, :], in0=z1,
                                     in1=t("e1_m1")[:, :])

            def e1ln(_=None):
                nc.vector.ln_bwd_dx(t("e1")[:, :], z1, z2, col(C_NA1E1),
                                    col(C_NBE1))

            def sq(i):      # ACT squares: 0 <- e0, 1 <- e1, 2 <- z2
                src = {0: t("e0"), 1: t("e1")}.get(i)
                scol = {0: C_SQ0S, 1: C_SQ1S, 2: C_SQ2S}[i]
                dst = t(f"sq{i}")
                if i == 2:
                    nc.scalar.activation(dst[:, :], z2, AF.Square,
                                         bias=col(C_SQ2B), scale=col(scol))
                else:
                    nc.scalar.activation(dst[:, :], src[:, :], AF.Square,
                                         scale=col(scol))

            def d1(_=None):
                E("d1").tensor_add(out=t("d1")[:, :], in0=t("sq1")[:, :],
                                   in1=t("sq2")[:, :])

            def d2(_=None):
                E("d2").tensor_add(out=t("d2")[:, :], in0=t("d1")[:, :],
                                   in1=t("sq0")[:, :])

            def krbf(_=None):
                nc.scalar.activation(t("krbf")[:, :], t("d2")[:, :], AF.Exp,
                                     bias=col(C_BETA0), scale=col(C_NEGG))

            def p2(_=None):
                nc.scalar.activation(t("p2")[:, :], t("kl")[:, :], AF.Square,
                                     bias=col(C_P2B), scale=col(C_P2S))

            def tpoly(_=None):
                nc.vector.scalar_tensor_tensor(t("tpoly")[:, :],
                                               t("kl")[:, :], col(C_W1U0),
                                               t("p2")[:, :], OP.mult, op_w2)

            def tpm(_=None):
                ts_op("tpm", t("tp_m")[:, :], t("kl")[:, :], col(C_W1U0), 0.0)

            def tpa(_=None):
                if sw2_pos:
                    E("tpa").tensor_add(out=t("tpoly")[:, :],
                                        in0=t("tp_m")[:, :],
                                        in1=t("p2")[:, :])
                else:
                    E("tpa").tensor_sub(out=t("tpoly")[:, :],
                                        in0=t("tp_m")[:, :],
                                        in1=t("p2")[:, :])

            def t2(_=None):
                if sw0_pos:
                    E("t2").tensor_add(out=t("t2")[:, :],
                                       in0=t("tpoly")[:, :],
                                       in1=t("krbf")[:, :])
                else:
                    E("t2").tensor_sub(out=t("t2")[:, :],
                                       in0=t("tpoly")[:, :],
                                       in1=t("krbf")[:, :])

            def th(_=None):
                nc.scalar.activation(t("th")[:, :], t("t2")[:, :], AF.Tanh,
                                     scale=0.5)

            def g1(o):
                if ybatch:
                    dst = yslice("gcat", f"y{o}")
                else:
                    dst = t(f"g1{o}")[:, :]
                if o in g1_act:
                    nc.scalar.activation(dst, t("th")[:, :],
                                         AF.Identity, bias=col(C_G1B + o),
                                         scale=col(C_G1S + o))
                else:
                    E(f"g1{o}").tensor_scalar(dst,
                                              t("th")[:, :], col(C_G1S + o),
                                              col(C_G1B + o), OP.mult,
                                              OP.add)

            def v(o, nsplit=1):
                vt = t(f"v{o}")
                cw = FREE // nsplit
                for sdx in range(nsplit):
                    sl = (slice(None), slice(sdx * cw, (sdx + 1) * cw))
                    if ybatch:
                        yin = t("ycat", 3 * FREE)[
                            :, o * FREE + sdx * cw:o * FREE + (sdx + 1) * cw]
                        gin = t("gcat", 3 * FREE)[
                            :, o * FREE + sdx * cw:o * FREE + (sdx + 1) * cw]
                    else:
                        yin = t(f"y{o}")[sl]
                        gin = t(f"g1{o}")[sl]
                    E(f"v{o}").tensor_mul(out=vt[sl], in0=yin, in1=gin)
                    fin = vt
                    if not bo_zero:
                        fin = t(f"f{o}")
                        nc.vector.tensor_scalar(fin[sl], vt[sl], 1.0,
                                                col(C_BO + o), OP.mult,
                                                OP.add)
                    dmaeng(out_eng[o]).dma_start(out=outs[o][sl],
                                                 in_=fin[sl])

            # ---------- emission schedule ----------
            # Pool-assigned mul ops are emitted when their step comes up;
            # engine in-order sequencing follows emission order per engine.
            steps = {
                "e1m": e1m, "e1a": e1a, "e1ln": e1ln, "d1": d1, "d2": d2,
                "krbf": krbf, "p2": p2, "tpoly": tpoly, "tpm": tpm,
                "tpa": tpa, "t2": t2, "th": th,
                "ya1b": ya1b, "ya2b": ya2b,
            }
            for f in FORM:
                steps[f + "m1"] = (lambda ff: lambda _=None: m1(ff))(f)
                steps[f + "a1"] = (lambda ff: lambda _=None: a1(ff))(f)
                steps[f + "ln1"] = (lambda ff: lambda _=None: ln1(ff))(f)
                steps[f + "m2"] = (lambda ff: lambda _=None: m2(ff))(f)
                steps[f + "a2"] = (lambda ff: lambda _=None: a2(ff))(f)
                steps[f + "s2"] = (lambda ff: lambda _=None: s2(ff))(f)
            for o in range(3):
                steps[f"sq{o}"] = (lambda oo: lambda _=None: sq(oo))(o)
                steps[f"g1{o}"] = (lambda oo: lambda _=None: g1(oo))(o)
                steps[f"v{o}"] = (lambda oo: lambda _=None: v(oo))(o)
                steps[f"v{o}s"] = (lambda oo: lambda _=None: v(oo, 2))(o)

            if dve_order is None:
                dve_order = DEFAULT_ORDER
            for s in dve_order:
                steps[s]()
    nc.compile()
    return nc


# Default schedule: e1 first (z2 arrives early on pool queue), kl path
# early (feeds p2 before krbf), e0 path, squares interleave on ACT, y
# mac work fills DVE while ACT runs, tail g/v lanes. Pool-assigned ops
# appear in the order too (per-engine in-order follows emission order).
DEFAULT_ORDER = (
    "klm1", "e0m1", "y0m1",           # dve ts of z1 (start asap)
    "sq2", "e1m", "y0m2",             # act: z2 square + offloaded affines
    "e0m2", "klm2", "y1m1", "y2m1", "y1m2", "y2m2",   # pool ts queue
    "kla1", "e1a", "e0a1",
    "sq1",
    "kla2", "e0a2",
    "sq0",
    "y0a1", "d1", "d2",
    "p2", "krbf",
    "y1a1", "tpoly", "t2",
    "th",
    "y0a2", "y1a2", "y2a1", "y2a2",
    "g10", "g11", "g12",
    "v0", "v1", "v2",
)


def _get_nc(sw0_pos, sw2_pos, bo_zero, cfg=None):
    def freeze(v):
        if isinstance(v, dict):
            return tuple(sorted(v.items()))
        return v
    key = (sw0_pos, sw2_pos, bo_zero,
           tuple(sorted((k, freeze(v)) for k, v in (cfg or {}).items())))
    if key not in _NC_CACHE:
        _NC_CACHE[key] = _build_nc(sw0_pos, sw2_pos, bo_zero, cfg)
    return _NC_CACHE[key]


def _host_prep(inputs):
    d = {k: np.asarray(v, dtype=np.float64) for k, v in inputs.items()}
    z = np.asarray(inputs["z"], dtype=np.float32)
    B, C, H, W = z.shape
    Wz, bz = d["z_proj_w"], d["z_proj_b"]
    Wt, bt = d["text_proj_w"], d["text_proj_b"]
    Wo, bo = d["out_w"], d["out_b"]
    gamma = np.exp(d["log_gamma"])
    alpha, c, w = d["alpha"], d["c"], d["w"]
    sumw = w.sum() + 1e-8
    w0p, w1p, w2p = w[0] / sumw, w[1] / sumw, w[2] / sumw

    t = d["text_vec"] @ Wt.T + bt                      # [B, HID]
    G = Wz.T @ Wz
    L = np.linalg.cholesky(G)                          # may raise
    delta = bz[None, :] - t
    v = delta @ Wz                                     # [B, 3]
    cdist = (delta ** 2).sum(1)
    r = np.linalg.solve(L, v.T).T                      # [B, 3]
    rho = cdist - (r ** 2).sum(1)
    u = t @ Wz                                         # [B, 3]
    s = t @ bz                                         # [B]
    M = Wo @ Wz                                        # [3, 3]
    m = Wo @ bz                                        # [3]

    u0 = u[:, 0]
    piv = [L[0, 0], L[1, 1]] + [M[o, 0] for o in range(3)]
    if min(abs(np.asarray(piv))) < 1e-7 or np.any(np.abs(u0) < 1e-7):
        raise np.linalg.LinAlgError("degenerate pivot")

    if w0p == 0.0:
        beta0 = np.full(B, -1e30)
    else:
        beta0 = -gamma * rho + np.log(abs(w0p))
    sw2 = np.sqrt(abs(w2p))

    cb = np.zeros((B, NCONST), dtype=np.float64)
    cb[:, C_A1E0] = L[1, 0] / L[0, 0]
    cb[:, C_BE0] = r[:, 0] / L[0, 0]
    cb[:, C_A2E0] = L[2, 0] / L[0, 0]
    cb[:, C_SQ0S] = L[0, 0]
    cb[:, C_A1E1] = L[2, 1] / L[1, 1]
    cb[:, C_BE1] = r[:, 1] / L[1, 1]
    cb[:, C_SQ1S] = L[1, 1]
    cb[:, C_SQ2S] = L[2, 2]
    cb[:, C_SQ2B] = r[:, 2]
    cb[:, C_NEGG] = -gamma
    cb[:, C_BETA0] = beta0
    cb[:, C_A1KL] = u[:, 1] / u0
    cb[:, C_BKL] = s / u0
    cb[:, C_A2KL] = u[:, 2] / u0
    cb[:, C_P2S] = alpha * sw2 * u0
    cb[:, C_P2B] = c * sw2
    cb[:, C_W1U0] = w1p * u0
    for o in range(3):
        cb[:, C_A1Y + o] = M[o, 1] / M[o, 0]
        cb[:, C_BY + o] = m[o] / M[o, 0]
        cb[:, C_A2Y + o] = M[o, 2] / M[o, 0]
        cb[:, C_G1S + o] = 0.5 * M[o, 0]
        cb[:, C_G1B + o] = 1.5 * M[o, 0]
        cb[:, C_BO + o] = bo[o]
    # negated copies for ln_bwd (out = in0 - in1*s0 - s1)
    cb[:, C_NA1E0] = -cb[:, C_A1E0]
    cb[:, C_NBE0] = -cb[:, C_BE0]
    cb[:, C_NA1E1] = -cb[:, C_A1E1]
    cb[:, C_NBE1] = -cb[:, C_BE1]
    cb[:, C_NA1KL] = -cb[:, C_A1KL]
    cb[:, C_NBKL] = -cb[:, C_BKL]
    for o in range(3):
        cb[:, C_NA1Y + o] = -cb[:, C_A1Y + o]
        cb[:, C_NBY + o] = -cb[:, C_BY + o]
    cb = cb.astype(np.float32)

    z16 = z.astype(np.float16)
    in_maps = []
    for core in range(NCORES):
        cs = np.empty((P, NCONST), dtype=np.float32)
        z01a = np.empty((P, 2 * FREE), dtype=np.float16)
        z2a = np.empty((P, FREE), dtype=np.float16)
        for j in range(BPC):
            b = core * BPC + j
            pl = z16[b].reshape(3, ROWS, FREE)
            rs = slice(j * ROWS, (j + 1) * ROWS)
            z01a[rs, 0:FREE] = pl[1]
            z01a[rs, FREE:2 * FREE] = pl[0]
            z2a[rs, :] = pl[2]
            cs[rs, :] = cb[b]
        in_maps.append({"consts": cs, "z01": z01a, "z2": z2a})
    flags = (bool(w0p >= 0.0), bool(w2p >= 0.0),
             bool(np.all(bo == 0.0)))
    return in_maps, flags, (B, C, H, W)


def _numpy_fallback(inputs):
    d = {k: np.asarray(v, dtype=np.float64) for k, v in inputs.items()}
    z, Wz, bz = d["z"], d["z_proj_w"], d["z_proj_b"]
    t = d["text_vec"] @ d["text_proj_w"].T + d["text_proj_b"]
    zm = np.einsum("bchw,oc->bohw", z, Wz) + bz[None, :, None, None]
    gamma = np.exp(d["log_gamma"])
    diff = zm - t[:, :, None, None]
    dist = (diff * diff).sum(1)
    klin = np.einsum("bchw,bc->bhw", zm, t)
    krbf = np.exp(-gamma * dist)
    kpoly = (d["alpha"] * klin + d["c"]) ** 2
    w = d["w"]
    k = (w[0] * krbf + w[1] * klin + w[2] * kpoly) / (w.sum() + 1e-8)
    zf = zm * (1.0 + 1.0 / (1.0 + np.exp(-k[:, None])))
    out = np.einsum("bchw,oc->bohw", zf, d["out_w"]) + d["out_b"][None, :, None, None]
    return out.astype(np.float32)


def run(inputs, trace=False, cfg=None):
    if cfg is None:
        cfg = BEST_CFG
    try:
        in_maps, (sw0, sw2, boz), (B, C, H, W) = _host_prep(inputs)
    except np.linalg.LinAlgError:
        return _numpy_fallback(inputs), None
    nc = _get_nc(sw0, sw2, boz, cfg)
    res = bass_utils.run_bass_kernel_spmd(
        nc, in_maps, core_ids=list(range(NCORES)), trace=trace)
    out = np.empty((B, C, H, W), dtype=np.float32)
    for core in range(NCORES):
        r = res.results[core]
        for j in range(BPC):
            b = core * BPC + j
            rs = slice(j * ROWS, (j + 1) * ROWS)
            for o in range(3):
                out[b, o] = np.asarray(r[f"o{o}"][rs, :],
                                       dtype=np.float32).reshape(H, W)
    return out, res


def kernel(**inputs):
    out, _ = run(inputs, trace=False)
    return out


# revision 18
# speedup vs baseline: 1.0120x; 1.0120x over previous
"""Fused per-pixel kernel for nn_KernelFusion_19026705121450 on 8 trn2 cores.

Math: per pixel q = z[b,:,h,w] (3 ch), per batch t = Wt text + bt:
    z_map = Wz q + bz; dist = ||z_map - t||^2; kl = z_map . t
    k = (w0 e^{-g dist} + w1 kl + w2 (a kl + c)^2) / (sum w + 1e-8)
    out = Wo (z_map (1 + sigmoid(k))) + bo

All 64-dim reductions collapse (host, fp64) to 3-dim forms:
    dist = ||L^T q + r||^2 + rho   (L = chol(Wz^T Wz))
    kl   = u . q + s
    out_o = (M_o . q + m_o) g + bo_o,  M = Wo Wz, g = 1.5 + 0.5 tanh(k/2)

Device: one 1024-col pass over [128, 1024] fp16 tiles (partition =
batch*64 + rowblock, free = pixel). Forms are pivot-normalized on their
lead channel so biases ride tensor_scalar const slots; pivot scales
refold into ACT Square scales / per-o g1 consts. tanh (same ACT table
as exp/square) replaces sigmoid to avoid a table reload; a warmup ACT
op preloads the table before DMAs land. MACs decompose per cfg across
DVE (ts+tt / stt / ln_bwd custom op) and Pool (ts half).
"""

import sys

if "/opt/trn_rl_repo" not in sys.path:
    sys.path.insert(0, "/opt/trn_rl_repo")

import numpy as np

import concourse.bass as bass
import concourse.bacc as bacc
import concourse.mybir as mybir
from concourse.tile import TileContext
from concourse import bass_utils

F32 = mybir.dt.float32
F16 = mybir.dt.float16
AF = mybir.ActivationFunctionType
OP = mybir.AluOpType

NCORES = 8
BPC = 2          # batches per core
ROWS = 64        # partition rows per batch
P = 128
FREE = 1024

# const column indices (fp32 tensor)
# form f: z_lead + a1*z_a + a2*z_b + bias  (negated copies for ln mode)
C_A1E0, C_BE0, C_A2E0, C_SQ0S = 0, 1, 2, 3
C_A1E1, C_BE1, C_SQ1S = 4, 5, 6
C_SQ2S, C_SQ2B = 7, 8
C_NEGG, C_BETA0 = 9, 10
C_A1KL, C_BKL, C_A2KL = 11, 12, 13
C_P2S, C_P2B, C_W1U0 = 14, 15, 16
C_A1Y, C_BY, C_A2Y = 17, 20, 23       # +o
C_G1S, C_G1B = 26, 29                 # +o
C_BO = 32                             # +o
C_NA1E0, C_NBE0 = 35, 36              # negated (for ln_bwd mode)
C_NA1E1, C_NBE1 = 37, 38
C_NA1KL, C_NBKL = 39, 40
C_NA1Y, C_NBY = 41, 44                # +o
C_THS, C_THB = 47, 48                 # tanh scale/bias (poly-square form)
NCONST = 49

_NC_CACHE: dict = {}

# Best found schedule: dist path first (krbf fires early), y-form work
# fills DVE afterward, y0 muls ride ACT idle slots, outputs on the SP
# hardware-DGE queue. Measured 20661 ns in TimelineSim (8-core SPMD).
ORDER_BEST = (
    "klm1", "e0m1", "e1m",
    "e0m2", "klm2", "y1m1", "y2m1", "y1m2", "y2m2",
    "sq2",
    "e0a1", "kla1", "e1a",
    "sq1",
    "e0a2", "kla2",
    "sq0",
    "tpm", "d1", "d2",
    "p2", "krbf",
    "tpa", "t2",
    "th",
    "y0m1", "y0m2", "y0a1", "y1a1",
    "g11", "g12",
    "y0a2", "y1a2", "y2a1", "y2a2",
    "g10", "v0", "v1", "v2",
)

BEST_CFG: dict = {"in_eng": ("act", "sync", "sync", "sync"),
                  "zorder": "z2split", "act_ops": ("y0m1", "y0m2"),
                  "out_eng": ("sync", "sync", "sync"),
                  "dve_order": ORDER_BEST}


def _build_nc(sw0_pos: bool, sw2_pos: bool, bo_zero: bool, cfg: dict | None,
              polysq: bool = True):
    """Emission order is hand-scheduled for the in-order engines.

    Step names (used by the `plan` cfg: list of (step, engine) pairs, where
    engine is 'dve'|'pool'|'act' for compute placement where it matters):
      mul ops ("<form>m1" = ts of z_aux w/ bias, "<form>m2" = ts of z2),
      adds ("<form>a1" lead+m1, "<form>a2" +m2), squares/exp/tanh on ACT
      fixed, d1/d2/t2 adds, tpoly stt, g1_o, v_o.
    """
    cfg = dict(cfg or {})
    warm = cfg.get("warm", True)
    # consts, z1, z0, z2 DMA queues
    in_eng = cfg.get("in_eng", ("act", "sync", "sync", "pool"))
    out_eng = cfg.get("out_eng", ("sync", "pool", "act"))
    # placement of the movable mul/aux ops
    pool_ops = set(cfg.get("pool_ops",
                           ("e0m2", "klm2", "y1m1", "y2m1", "y1m2",
                            "y2m2")))
    act_ops = set(cfg.get("act_ops", ("e1m1", "y0m2")))
    dve_order = cfg.get("dve_order", None)
    g1_act = set(cfg.get("g1_act", (1, 2)))     # g1 indices on ACT

    nc = bacc.Bacc("TRN2", target_bir_lowering=False)
    cons = nc.dram_tensor("consts", [P, NCONST], F32, kind="ExternalInput")
    z01 = nc.dram_tensor("z01", [P, 2 * FREE], F16, kind="ExternalInput")
    z2d = nc.dram_tensor("z2", [P, FREE], F16, kind="ExternalInput")
    outs = [nc.dram_tensor(f"o{o}", [P, FREE], F16, kind="ExternalOutput")
            for o in range(3)]

    op_w2 = OP.add if sw2_pos else OP.subtract

    def dmaeng(which):
        return {"sync": nc.sync, "pool": nc.gpsimd, "act": nc.scalar,
                "dve": nc.vector}[which]

    with TileContext(nc) as tc:
        with tc.tile_pool(name="cpool", bufs=1) as cpool, \
             tc.tile_pool(name="work", bufs=1) as pool:
            ct = cpool.tile([P, NCONST], F32, name="ct")
            zt = cpool.tile([P, 2 * FREE], F16, name="zt")
            z2t = cpool.tile([P, FREE], F16, name="z2t")
            wt = cpool.tile([P, 1], F32, name="wt")

            if warm:
                nc.vector.memset(wt[:, :], 0.0)
                nc.scalar.activation(wt[:, :], wt[:, :], AF.Square)

            # packed z01 = [z1 | z0]
            zorder = cfg.get("zorder", "z2first")
            dmaeng(in_eng[0]).dma_start(out=ct[:, :], in_=cons[:, :])
            if zorder == "z2first":
                dmaeng(in_eng[3]).dma_start(out=z2t[:, :], in_=z2d[:, :])
                dmaeng(in_eng[1]).dma_start(out=zt[:, :], in_=z01[:, :])
            elif zorder == "z01first":
                dmaeng(in_eng[1]).dma_start(out=zt[:, :], in_=z01[:, :])
                dmaeng(in_eng[3]).dma_start(out=z2t[:, :], in_=z2d[:, :])
            elif zorder == "split":
                dmaeng(in_eng[1]).dma_start(out=zt[:, 0:FREE],
                                            in_=z01[:, 0:FREE])
                dmaeng(in_eng[3]).dma_start(out=z2t[:, :], in_=z2d[:, :])
                dmaeng(in_eng[2]).dma_start(out=zt[:, FREE:2 * FREE],
                                            in_=z01[:, FREE:2 * FREE])
            elif zorder == "z2split":  # z2, z1, z0 (all split)
                dmaeng(in_eng[3]).dma_start(out=z2t[:, :], in_=z2d[:, :])
                dmaeng(in_eng[1]).dma_start(out=zt[:, 0:FREE],
                                            in_=z01[:, 0:FREE])
                dmaeng(in_eng[2]).dma_start(out=zt[:, FREE:2 * FREE],
                                            in_=z01[:, FREE:2 * FREE])
            else:  # z1first: z1, z2, z0
                dmaeng(in_eng[1]).dma_start(out=zt[:, 0:FREE],
                                            in_=z01[:, 0:FREE])
                dmaeng(in_eng[3]).dma_start(out=z2t[:, :], in_=z2d[:, :])
                dmaeng(in_eng[2]).dma_start(out=zt[:, FREE:2 * FREE],
                                            in_=z01[:, FREE:2 * FREE])
            z1 = zt[:, 0:FREE]
            z0 = zt[:, FREE:2 * FREE]
            z2 = z2t[:, :]

            def col(j):
                return ct[:, j:j + 1]

            tiles = {}

            def t(tag, w=FREE):
                if tag not in tiles:
                    tiles[tag] = pool.tile([P, w], F16, tag=tag, name=tag)
                return tiles[tag]

            def E(tag):
                return nc.gpsimd if tag in pool_ops else nc.vector

            def ts_op(tag, dst, src, scol, bcol):
                # dst = scol*src + bcol on pool/act/dve per placement
                if tag in act_ops:
                    nc.scalar.activation(dst, src, AF.Identity,
                                         bias=bcol if not isinstance(bcol, float)
                                         else bcol, scale=scol)
                else:
                    E(tag).tensor_scalar(dst, src, scol, bcol,
                                         OP.mult, OP.add)

            # ---------- op emitters (callable in any order) ----------
            # forms: f in {e0, e1, kl, y0, y1, y2}
            # e1 is 2-term: lead z1, aux z2. others: lead z0, aux z1 + z2.
            FORM = {
                "e0": (None, C_A1E0, C_BE0, C_A2E0, C_NA1E0, C_NBE0),
                "kl": (None, C_A1KL, C_BKL, C_A2KL, C_NA1KL, C_NBKL),
                "y0": (None, C_A1Y + 0, C_BY + 0, C_A2Y + 0, C_NA1Y, C_NBY),
                "y1": (None, C_A1Y + 1, C_BY + 1, C_A2Y + 1, C_NA1Y + 1,
                       C_NBY + 1),
                "y2": (None, C_A1Y + 2, C_BY + 2, C_A2Y + 2, C_NA1Y + 2,
                       C_NBY + 2),
            }

            ybatch = cfg.get("ybatch", False)

            def yslice(base, f):
                o = int(f[1])
                return t(base, 3 * FREE)[:, o * FREE:(o + 1) * FREE]

            def m1(f):      # tmp = a1*z1 + b   (aux mul with bias)
                _, a1, b, _, _, _ = FORM[f]
                if ybatch and f.startswith("y"):
                    dst = yslice("ymcat", f)
                else:
                    dst = t(f + "_m1")[:, :]
                ts_op(f + "m1", dst, z1, col(a1), col(b))

            def a1(f):      # acc = z0 + tmp
                nc.vector.tensor_add(out=t(f + "_a")[:, :], in0=z0,
                                     in1=t(f + "_m1")[:, :])

            def ln1(f):     # acc = z0 + a1*z1 + b via custom op (1 DVE op)
                _, _, _, _, na1, nb = FORM[f]
                nc.vector.ln_bwd_dx(t(f + "_a")[:, :], z0, z1, col(na1),
                                    col(nb))

            def m2(f):      # tmp2 = a2*z2
                _, _, _, a2, _, _ = FORM[f]
                if ybatch and f.startswith("y"):
                    dst = yslice("ym2cat", f)
                else:
                    dst = t(f + "_m2")[:, :]
                ts_op(f + "m2", dst, z2, col(a2), 0.0)

            def ya1b(_=None):   # all three y a1 joins in one wide op
                mc = t("ymcat", 3 * FREE)[:, :].rearrange(
                    "p (o f) -> p o f", o=3)
                ac = t("yacat", 3 * FREE)[:, :].rearrange(
                    "p (o f) -> p o f", o=3)
                zb = z0.unsqueeze(1).broadcast_to([P, 3, FREE])
                nc.vector.tensor_add(out=ac, in0=zb, in1=mc)

            def ya2b(_=None):   # all three y a2 joins in one wide op
                nc.vector.tensor_add(out=t("ycat", 3 * FREE)[:, :],
                                     in0=t("yacat", 3 * FREE)[:, :],
                                     in1=t("ym2cat", 3 * FREE)[:, :])

            def a2(f):      # out = acc + tmp2
                E(f + "a2").tensor_add(out=t(f)[:, :], in0=t(f + "_a")[:, :],
                                       in1=t(f + "_m2")[:, :])

            def s2(f):      # out = acc + a2*z2 via stt (skip m2)
                _, _, _, a2c, _, _ = FORM[f]
                nc.vector.scalar_tensor_tensor(t(f)[:, :], z2, col(a2c),
                                               t(f + "_a")[:, :], OP.mult,
                                               OP.add)

            def e1m(_=None):   # e1 aux: tmp = a1*z2 + b
                ts_op("e1m1", t("e1_m1")[:, :], z2, col(C_A1E1),
                      col(C_BE1))

            def e1a(_=None):   # e1 = z1 + tmp
                nc.vector.tensor_add(out=t("e1")[:, :], in0=z1,
                                     in1=t("e1_m1")[:, :])

            def e1ln(_=None):
                nc.vector.ln_bwd_dx(t("e1")[:, :], z1, z2, col(C_NA1E1),
                                    col(C_NBE1))

            def sq(i):      # ACT squares: 0 <- e0, 1 <- e1, 2 <- z2
                src = {0: t("e0"), 1: t("e1")}.get(i)
                scol = {0: C_SQ0S, 1: C_SQ1S, 2: C_SQ2S}[i]
                dst = t(f"sq{i}")
                if i == 2:
                    nc.scalar.activation(dst[:, :], z2, AF.Square,
                                         bias=col(C_SQ2B), scale=col(scol))
                else:
                    nc.scalar.activation(dst[:, :], src[:, :], AF.Square,
                                         scale=col(scol))

            def d1(_=None):
                E("d1").tensor_add(out=t("d1")[:, :], in0=t("sq1")[:, :],
                                   in1=t("sq2")[:, :])

            def d2(_=None):
                E("d2").tensor_add(out=t("d2")[:, :], in0=t("d1")[:, :],
                                   in1=t("sq0")[:, :])

            def krbf(_=None):
                nc.scalar.activation(t("krbf")[:, :], t("d2")[:, :], AF.Exp,
                                     bias=col(C_BETA0), scale=col(C_NEGG))

            def p2(_=None):
                nc.scalar.activation(t("p2")[:, :], t("kl")[:, :], AF.Square,
                                     bias=col(C_P2B), scale=col(C_P2S))

            def tpoly(_=None):
                nc.vector.scalar_tensor_tensor(t("tpoly")[:, :],
                                               t("kl")[:, :], col(C_W1U0),
                                               t("p2")[:, :], OP.mult, op_w2)

            def tpm(_=None):
                ts_op("tpm", t("tp_m")[:, :], t("kl")[:, :], col(C_W1U0), 0.0)

            def tpa(_=None):
                if sw2_pos:
                    E("tpa").tensor_add(out=t("tpoly")[:, :],
                                        in0=t("tp_m")[:, :],
                                        in1=t("p2")[:, :])
                else:
                    E("tpa").tensor_sub(out=t("tpoly")[:, :],
                                        in0=t("tp_m")[:, :],
                                        in1=t("p2")[:, :])

            def t2(_=None):
                if polysq:
                    # poly+linear completed into p2; k = sA*(p2 +/- krbf)+k0
                    src = t("p2")
                    op_add = (sw0_pos == sw2_pos)
                else:
                    src = t("tpoly")
                    op_add = sw0_pos
                if op_add:
                    E("t2").tensor_add(out=t("t2")[:, :], in0=src[:, :],
                                       in1=t("krbf")[:, :])
                else:
                    E("t2").tensor_sub(out=t("t2")[:, :], in0=src[:, :],
                                       in1=t("krbf")[:, :])

            def th(_=None):
                if polysq:
                    nc.scalar.activation(t("th")[:, :], t("t2")[:, :],
                                         AF.Tanh, scale=col(C_THS),
                                         bias=col(C_THB))
                else:
                    nc.scalar.activation(t("th")[:, :], t("t2")[:, :],
                                         AF.Tanh, scale=0.5)

            def g1(o):
                if ybatch:
                    dst = yslice("gcat", f"y{o}")
                else:
                    dst = t(f"g1{o}")[:, :]
                if o in g1_act:
                    nc.scalar.activation(dst, t("th")[:, :],
                                         AF.Identity, bias=col(C_G1B + o),
                                         scale=col(C_G1S + o))
                else:
                    E(f"g1{o}").tensor_scalar(dst,
                                              t("th")[:, :], col(C_G1S + o),
                                              col(C_G1B + o), OP.mult,
                                              OP.add)

            def v(o, nsplit=1):
                vt = t(f"v{o}")
                cw = FREE // nsplit
                for sdx in range(nsplit):
                    sl = (slice(None), slice(sdx * cw, (sdx + 1) * cw))
                    if ybatch:
                        yin = t("ycat", 3 * FREE)[
                            :, o * FREE + sdx * cw:o * FREE + (sdx + 1) * cw]
                        gin = t("gcat", 3 * FREE)[
                            :, o * FREE + sdx * cw:o * FREE + (sdx + 1) * cw]
                    else:
                        yin = t(f"y{o}")[sl]
                        gin = t(f"g1{o}")[sl]
                    E(f"v{o}").tensor_mul(out=vt[sl], in0=yin, in1=gin)
                    fin = vt
                    if not bo_zero:
                        fin = t(f"f{o}")
                        nc.vector.tensor_scalar(fin[sl], vt[sl], 1.0,
                                                col(C_BO + o), OP.mult,
                                                OP.add)
                    dmaeng(out_eng[o]).dma_start(out=outs[o][sl],
                                                 in_=fin[sl])

            # ---------- emission schedule ----------
            # Pool-assigned mul ops are emitted when their step comes up;
            # engine in-order sequencing follows emission order per engine.
            steps = {
                "e1m": e1m, "e1a": e1a, "e1ln": e1ln, "d1": d1, "d2": d2,
                "krbf": krbf, "p2": p2, "tpoly": tpoly, "tpm": tpm,
                "tpa": tpa, "t2": t2, "th": th,
                "ya1b": ya1b, "ya2b": ya2b,
            }
            for f in FORM:
                steps[f + "m1"] = (lambda ff: lambda _=None: m1(ff))(f)
                steps[f + "a1"] = (lambda ff: lambda _=None: a1(ff))(f)
                steps[f + "ln1"] = (lambda ff: lambda _=None: ln1(ff))(f)
                steps[f + "m2"] = (lambda ff: lambda _=None: m2(ff))(f)
                steps[f + "a2"] = (lambda ff: lambda _=None: a2(ff))(f)
                steps[f + "s2"] = (lambda ff: lambda _=None: s2(ff))(f)
            for o in range(3):
                steps[f"sq{o}"] = (lambda oo: lambda _=None: sq(oo))(o)
                steps[f"g1{o}"] = (lambda oo: lambda _=None: g1(oo))(o)
                steps[f"v{o}"] = (lambda oo: lambda _=None: v(oo))(o)
                steps[f"v{o}s"] = (lambda oo: lambda _=None: v(oo, 2))(o)

            if dve_order is None:
                dve_order = DEFAULT_ORDER
            for s in dve_order:
                if polysq and s in ("tpm", "tpa", "tpoly"):
                    continue
                steps[s]()
    nc.compile()
    return nc


# Default schedule: e1 first (z2 arrives early on pool queue), kl path
# early (feeds p2 before krbf), e0 path, squares interleave on ACT, y
# mac work fills DVE while ACT runs, tail g/v lanes. Pool-assigned ops
# appear in the order too (per-engine in-order follows emission order).
DEFAULT_ORDER = (
    "klm1", "e0m1", "y0m1",           # dve ts of z1 (start asap)
    "sq2", "e1m", "y0m2",             # act: z2 square + offloaded affines
    "e0m2", "klm2", "y1m1", "y2m1", "y1m2", "y2m2",   # pool ts queue
    "kla1", "e1a", "e0a1",
    "sq1",
    "kla2", "e0a2",
    "sq0",
    "y0a1", "d1", "d2",
    "p2", "krbf",
    "y1a1", "tpoly", "t2",
    "th",
    "y0a2", "y1a2", "y2a1", "y2a2",
    "g10", "g11", "g12",
    "v0", "v1", "v2",
)


def _get_nc(sw0_pos, sw2_pos, bo_zero, cfg=None, polysq=True):
    def freeze(v):
        if isinstance(v, dict):
            return tuple(sorted(v.items()))
        return v
    key = (sw0_pos, sw2_pos, bo_zero, polysq,
           tuple(sorted((k, freeze(v)) for k, v in (cfg or {}).items())))
    if key not in _NC_CACHE:
        _NC_CACHE[key] = _build_nc(sw0_pos, sw2_pos, bo_zero, cfg, polysq)
    return _NC_CACHE[key]


def _host_prep(inputs):
    d = {k: np.asarray(v, dtype=np.float64) for k, v in inputs.items()}
    z = np.asarray(inputs["z"], dtype=np.float32)
    B, C, H, W = z.shape
    Wz, bz = d["z_proj_w"], d["z_proj_b"]
    Wt, bt = d["text_proj_w"], d["text_proj_b"]
    Wo, bo = d["out_w"], d["out_b"]
    gamma = np.exp(d["log_gamma"])
    alpha, c, w = d["alpha"], d["c"], d["w"]
    sumw = w.sum() + 1e-8
    w0p, w1p, w2p = w[0] / sumw, w[1] / sumw, w[2] / sumw

    t = d["text_vec"] @ Wt.T + bt                      # [B, HID]
    G = Wz.T @ Wz
    L = np.linalg.cholesky(G)                          # may raise
    delta = bz[None, :] - t
    v = delta @ Wz                                     # [B, 3]
    cdist = (delta ** 2).sum(1)
    r = np.linalg.solve(L, v.T).T                      # [B, 3]
    rho = cdist - (r ** 2).sum(1)
    u = t @ Wz                                         # [B, 3]
    s = t @ bz                                         # [B]
    M = Wo @ Wz                                        # [3, 3]
    m = Wo @ bz                                        # [3]

    u0 = u[:, 0]
    piv = [L[0, 0], L[1, 1]] + [M[o, 0] for o in range(3)]
    if min(abs(np.asarray(piv))) < 1e-7 or np.any(np.abs(u0) < 1e-7):
        raise np.linalg.LinAlgError("degenerate pivot")

    if w0p == 0.0:
        beta0 = np.full(B, -1e30)
    else:
        beta0 = -gamma * rho + np.log(abs(w0p))
    sw2 = np.sqrt(abs(w2p))

    cb = np.zeros((B, NCONST), dtype=np.float64)
    cb[:, C_A1E0] = L[1, 0] / L[0, 0]
    cb[:, C_BE0] = r[:, 0] / L[0, 0]
    cb[:, C_A2E0] = L[2, 0] / L[0, 0]
    cb[:, C_SQ0S] = L[0, 0]
    cb[:, C_A1E1] = L[2, 1] / L[1, 1]
    cb[:, C_BE1] = r[:, 1] / L[1, 1]
    cb[:, C_SQ1S] = L[1, 1]
    cb[:, C_SQ2S] = L[2, 2]
    cb[:, C_SQ2B] = r[:, 2]
    cb[:, C_NEGG] = -gamma
    cb[:, C_BETA0] = beta0
    cb[:, C_A1KL] = u[:, 1] / u0
    cb[:, C_BKL] = s / u0
    cb[:, C_A2KL] = u[:, 2] / u0
    # completed square: w1p*kl + w2p*(alpha*kl+c)^2 = sA*(s*kl' + b)^2 + k0
    # with kl = u0*kl'; falls back to the tpoly path when degenerate.
    polysq = bool(abs(w2p) * alpha ** 2 > 1e-12)
    sign_a = 1.0 if w2p >= 0 else -1.0
    if polysq:
        sabs = alpha * sw2 * np.abs(u0)              # sqrt|A| per batch
        bp = w1p * u0 + 2.0 * w2p * alpha * c * u0
        bsq = bp * sign_a / (2.0 * sabs)
        kconst = w2p * c ** 2 - sign_a * bsq ** 2
        cb[:, C_P2S] = sabs
        cb[:, C_P2B] = bsq
        cb[:, C_THS] = 0.5 * sign_a
        cb[:, C_THB] = 0.5 * kconst
    else:
        cb[:, C_P2S] = alpha * sw2 * u0
        cb[:, C_P2B] = c * sw2
    cb[:, C_W1U0] = w1p * u0
    for o in range(3):
        cb[:, C_A1Y + o] = M[o, 1] / M[o, 0]
        cb[:, C_BY + o] = m[o] / M[o, 0]
        cb[:, C_A2Y + o] = M[o, 2] / M[o, 0]
        cb[:, C_G1S + o] = 0.5 * M[o, 0]
        cb[:, C_G1B + o] = 1.5 * M[o, 0]
        cb[:, C_BO + o] = bo[o]
    # negated copies for ln_bwd (out = in0 - in1*s0 - s1)
    cb[:, C_NA1E0] = -cb[:, C_A1E0]
    cb[:, C_NBE0] = -cb[:, C_BE0]
    cb[:, C_NA1E1] = -cb[:, C_A1E1]
    cb[:, C_NBE1] = -cb[:, C_BE1]
    cb[:, C_NA1KL] = -cb[:, C_A1KL]
    cb[:, C_NBKL] = -cb[:, C_BKL]
    for o in range(3):
        cb[:, C_NA1Y + o] = -cb[:, C_A1Y + o]
        cb[:, C_NBY + o] = -cb[:, C_BY + o]
    cb = cb.astype(np.float32)

    z16 = z.astype(np.float16)
    in_maps = []
    for core in range(NCORES):
        cs = np.empty((P, NCONST), dtype=np.float32)
        z01a = np.empty((P, 2 * FREE), dtype=np.float16)
        z2a = np.empty((P, FREE), dtype=np.float16)
        for j in range(BPC):
            b = core * BPC + j
            pl = z16[b].reshape(3, ROWS, FREE)
            rs = slice(j * ROWS, (j + 1) * ROWS)
            z01a[rs, 0:FREE] = pl[1]
            z01a[rs, FREE:2 * FREE] = pl[0]
            z2a[rs, :] = pl[2]
            cs[rs, :] = cb[b]
        in_maps.append({"consts": cs, "z01": z01a, "z2": z2a})
    flags = (bool(w0p >= 0.0), bool(w2p >= 0.0),
             bool(np.all(bo == 0.0)), polysq)
    return in_maps, flags, (B, C, H, W)


def _numpy_fallback(inputs):
    d = {k: np.asarray(v, dtype=np.float64) for k, v in inputs.items()}
    z, Wz, bz = d["z"], d["z_proj_w"], d["z_proj_b"]
    t = d["text_vec"] @ d["text_proj_w"].T + d["text_proj_b"]
    zm = np.einsum("bchw,oc->bohw", z, Wz) + bz[None, :, None, None]
    gamma = np.exp(d["log_gamma"])
    diff = zm - t[:, :, None, None]
    dist = (diff * diff).sum(1)
    klin = np.einsum("bchw,bc->bhw", zm, t)
    krbf = np.exp(-gamma * dist)
    kpoly = (d["alpha"] * klin + d["c"]) ** 2
    w = d["w"]
    k = (w[0] * krbf + w[1] * klin + w[2] * kpoly) / (w.sum() + 1e-8)
    zf = zm * (1.0 + 1.0 / (1.0 + np.exp(-k[:, None])))
    out = np.einsum("bchw,oc->bohw", zf, d["out_w"]) + d["out_b"][None, :, None, None]
    return out.astype(np.float32)


def run(inputs, trace=False, cfg=None):
    if cfg is None:
        cfg = BEST_CFG
    try:
        in_maps, (sw0, sw2, boz, psq), (B, C, H, W) = _host_prep(inputs)
    except np.linalg.LinAlgError:
        return _numpy_fallback(inputs), None
    nc = _get_nc(sw0, sw2, boz, cfg, psq)
    res = bass_utils.run_bass_kernel_spmd(
        nc, in_maps, core_ids=list(range(NCORES)), trace=trace)
    out = np.empty((B, C, H, W), dtype=np.float32)
    for core in range(NCORES):
        r = res.results[core]
        for j in range(BPC):
            b = core * BPC + j
            rs = slice(j * ROWS, (j + 1) * ROWS)
            for o in range(3):
                out[b, o] = np.asarray(r[f"o{o}"][rs, :],
                                       dtype=np.float32).reshape(H, W)
    return out, res


def kernel(**inputs):
    out, _ = run(inputs, trace=False)
    return out


# revision 19
# speedup vs baseline: 1.0197x; 1.0077x over previous
"""Fused per-pixel kernel for nn_KernelFusion_19026705121450 on 8 trn2 cores.

Math: per pixel q = z[b,:,h,w] (3 ch), per batch t = Wt text + bt:
    z_map = Wz q + bz; dist = ||z_map - t||^2; kl = z_map . t
    k = (w0 e^{-g dist} + w1 kl + w2 (a kl + c)^2) / (sum w + 1e-8)
    out = Wo (z_map (1 + sigmoid(k))) + bo

All 64-dim reductions collapse (host, fp64) to 3-dim forms:
    dist = ||L^T q + r||^2 + rho   (L = chol(Wz^T Wz))
    kl   = u . q + s
    out_o = (M_o . q + m_o) g + bo_o,  M = Wo Wz, g = 1.5 + 0.5 tanh(k/2)

Device: one 1024-col pass over [128, 1024] fp16 tiles (partition =
batch*64 + rowblock, free = pixel). Forms are pivot-normalized on their
lead channel so biases ride tensor_scalar const slots; pivot scales
refold into ACT Square scales / per-o g1 consts. tanh (same ACT table
as exp/square) replaces sigmoid to avoid a table reload; a warmup ACT
op preloads the table before DMAs land. MACs decompose per cfg across
DVE (ts+tt / stt / ln_bwd custom op) and Pool (ts half).
"""

import sys

if "/opt/trn_rl_repo" not in sys.path:
    sys.path.insert(0, "/opt/trn_rl_repo")

import numpy as np

import concourse.bass as bass
import concourse.bacc as bacc
import concourse.mybir as mybir
from concourse.tile import TileContext
from concourse import bass_utils

F32 = mybir.dt.float32
F16 = mybir.dt.float16
AF = mybir.ActivationFunctionType
OP = mybir.AluOpType

NCORES = 8
BPC = 2          # batches per core
ROWS = 64        # partition rows per batch
P = 128
FREE = 1024

# const column indices (fp32 tensor)
# form f: z_lead + a1*z_a + a2*z_b + bias  (negated copies for ln mode)
C_A1E0, C_BE0, C_A2E0, C_SQ0S = 0, 1, 2, 3
C_A1E1, C_BE1, C_SQ1S = 4, 5, 6
C_SQ2S, C_SQ2B = 7, 8
C_NEGG, C_BETA0 = 9, 10
C_A1KL, C_BKL, C_A2KL = 11, 12, 13
C_P2S, C_P2B, C_W1U0 = 14, 15, 16
C_A1Y, C_BY, C_A2Y = 17, 20, 23       # +o
C_G1S, C_G1B = 26, 29                 # +o
C_BO = 32                             # +o
C_NA1E0, C_NBE0 = 35, 36              # negated (for ln_bwd mode)
C_NA1E1, C_NBE1 = 37, 38
C_NA1KL, C_NBKL = 39, 40
C_NA1Y, C_NBY = 41, 44                # +o
C_THS, C_THB = 47, 48                 # tanh scale/bias (poly-square form)
NCONST = 49

_NC_CACHE: dict = {}

# Best found schedule: dist path first (krbf fires early), y-form work
# fills DVE afterward, y0 muls ride ACT idle slots, outputs on the SP
# hardware-DGE queue. Measured 20661 ns in TimelineSim (8-core SPMD).
ORDER_BEST = (
    "klm1", "e0m1", "e1m",
    "e0m2", "klm2", "y1m1", "y2m1", "y1m2", "y2m2",
    "sq2",
    "e0a1", "kla1", "e1a",
    "sq1",
    "e0a2", "kla2",
    "sq0",
    "tpm", "d1", "d2",
    "p2", "krbf",
    "tpa", "t2",
    "th",
    "y0m1", "y0m2", "y0a1", "y1a1",
    "g11", "g12",
    "y0a2", "y1a2", "y2a1", "y2a2",
    "g10", "v0", "v1", "v2",
)

BEST_CFG: dict = {"in_eng": ("act", "sync", "sync", "sync"),
                  "zorder": "z2split", "act_ops": ("y0m1", "y0m2"),
                  "out_eng": ("sync", "sync", "sync"),
                  "g1_act": (2,),
                  "dve_order": ORDER_BEST}


def _build_nc(sw0_pos: bool, sw2_pos: bool, bo_zero: bool, cfg: dict | None,
              polysq: bool = True):
    """Emission order is hand-scheduled for the in-order engines.

    Step names (used by the `plan` cfg: list of (step, engine) pairs, where
    engine is 'dve'|'pool'|'act' for compute placement where it matters):
      mul ops ("<form>m1" = ts of z_aux w/ bias, "<form>m2" = ts of z2),
      adds ("<form>a1" lead+m1, "<form>a2" +m2), squares/exp/tanh on ACT
      fixed, d1/d2/t2 adds, tpoly stt, g1_o, v_o.
    """
    cfg = dict(cfg or {})
    warm = cfg.get("warm", True)
    # consts, z1, z0, z2 DMA queues
    in_eng = cfg.get("in_eng", ("act", "sync", "sync", "pool"))
    out_eng = cfg.get("out_eng", ("sync", "pool", "act"))
    # placement of the movable mul/aux ops
    pool_ops = set(cfg.get("pool_ops",
                           ("e0m2", "klm2", "y1m1", "y2m1", "y1m2",
                            "y2m2")))
    act_ops = set(cfg.get("act_ops", ("e1m1", "y0m2")))
    dve_order = cfg.get("dve_order", None)
    g1_act = set(cfg.get("g1_act", (1, 2)))     # g1 indices on ACT

    nc = bacc.Bacc("TRN2", target_bir_lowering=False)
    cons = nc.dram_tensor("consts", [P, NCONST], F32, kind="ExternalInput")
    z01 = nc.dram_tensor("z01", [P, 2 * FREE], F16, kind="ExternalInput")
    z2d = nc.dram_tensor("z2", [P, FREE], F16, kind="ExternalInput")
    outs = [nc.dram_tensor(f"o{o}", [P, FREE], F16, kind="ExternalOutput")
            for o in range(3)]

    op_w2 = OP.add if sw2_pos else OP.subtract

    def dmaeng(which):
        return {"sync": nc.sync, "pool": nc.gpsimd, "act": nc.scalar,
                "dve": nc.vector}[which]

    with TileContext(nc) as tc:
        with tc.tile_pool(name="cpool", bufs=1) as cpool, \
             tc.tile_pool(name="work", bufs=1) as pool:
            ct = cpool.tile([P, NCONST], F32, name="ct")
            zt = cpool.tile([P, 2 * FREE], F16, name="zt")
            z2t = cpool.tile([P, FREE], F16, name="z2t")
            wt = cpool.tile([P, 1], F32, name="wt")

            if warm:
                nc.vector.memset(wt[:, :], 0.0)
                nc.scalar.activation(wt[:, :], wt[:, :], AF.Square)

            # packed z01 = [z1 | z0]
            zorder = cfg.get("zorder", "z2first")
            dmaeng(in_eng[0]).dma_start(out=ct[:, :], in_=cons[:, :])
            if zorder == "z2first":
                dmaeng(in_eng[3]).dma_start(out=z2t[:, :], in_=z2d[:, :])
                dmaeng(in_eng[1]).dma_start(out=zt[:, :], in_=z01[:, :])
            elif zorder == "z01first":
                dmaeng(in_eng[1]).dma_start(out=zt[:, :], in_=z01[:, :])
                dmaeng(in_eng[3]).dma_start(out=z2t[:, :], in_=z2d[:, :])
            elif zorder == "split":
                dmaeng(in_eng[1]).dma_start(out=zt[:, 0:FREE],
                                            in_=z01[:, 0:FREE])
                dmaeng(in_eng[3]).dma_start(out=z2t[:, :], in_=z2d[:, :])
                dmaeng(in_eng[2]).dma_start(out=zt[:, FREE:2 * FREE],
                                            in_=z01[:, FREE:2 * FREE])
            elif zorder == "z2split":  # z2, z1, z0 (all split)
                dmaeng(in_eng[3]).dma_start(out=z2t[:, :], in_=z2d[:, :])
                dmaeng(in_eng[1]).dma_start(out=zt[:, 0:FREE],
                                            in_=z01[:, 0:FREE])
                dmaeng(in_eng[2]).dma_start(out=zt[:, FREE:2 * FREE],
                                            in_=z01[:, FREE:2 * FREE])
            else:  # z1first: z1, z2, z0
                dmaeng(in_eng[1]).dma_start(out=zt[:, 0:FREE],
                                            in_=z01[:, 0:FREE])
                dmaeng(in_eng[3]).dma_start(out=z2t[:, :], in_=z2d[:, :])
                dmaeng(in_eng[2]).dma_start(out=zt[:, FREE:2 * FREE],
                                            in_=z01[:, FREE:2 * FREE])
            z1 = zt[:, 0:FREE]
            z0 = zt[:, FREE:2 * FREE]
            z2 = z2t[:, :]

            def col(j):
                return ct[:, j:j + 1]

            tiles = {}

            def t(tag, w=FREE):
                if tag not in tiles:
                    tiles[tag] = pool.tile([P, w], F16, tag=tag, name=tag)
                return tiles[tag]

            def E(tag):
                return nc.gpsimd if tag in pool_ops else nc.vector

            def ts_op(tag, dst, src, scol, bcol):
                # dst = scol*src + bcol on pool/act/dve per placement
                if tag in act_ops:
                    nc.scalar.activation(dst, src, AF.Identity,
                                         bias=bcol if not isinstance(bcol, float)
                                         else bcol, scale=scol)
                else:
                    E(tag).tensor_scalar(dst, src, scol, bcol,
                                         OP.mult, OP.add)

            # ---------- op emitters (callable in any order) ----------
            # forms: f in {e0, e1, kl, y0, y1, y2}
            # e1 is 2-term: lead z1, aux z2. others: lead z0, aux z1 + z2.
            FORM = {
                "e0": (None, C_A1E0, C_BE0, C_A2E0, C_NA1E0, C_NBE0),
                "kl": (None, C_A1KL, C_BKL, C_A2KL, C_NA1KL, C_NBKL),
                "y0": (None, C_A1Y + 0, C_BY + 0, C_A2Y + 0, C_NA1Y, C_NBY),
                "y1": (None, C_A1Y + 1, C_BY + 1, C_A2Y + 1, C_NA1Y + 1,
                       C_NBY + 1),
                "y2": (None, C_A1Y + 2, C_BY + 2, C_A2Y + 2, C_NA1Y + 2,
                       C_NBY + 2),
            }

            ybatch = cfg.get("ybatch", False)

            def yslice(base, f):
                o = int(f[1])
                return t(base, 3 * FREE)[:, o * FREE:(o + 1) * FREE]

            def m1(f):      # tmp = a1*z1 + b   (aux mul with bias)
                _, a1, b, _, _, _ = FORM[f]
                if ybatch and f.startswith("y"):
                    dst = yslice("ymcat", f)
                else:
                    dst = t(f + "_m1")[:, :]
                ts_op(f + "m1", dst, z1, col(a1), col(b))

            def a1(f):      # acc = z0 + tmp
                nc.vector.tensor_add(out=t(f + "_a")[:, :], in0=z0,
                                     in1=t(f + "_m1")[:, :])

            def ln1(f):     # acc = z0 + a1*z1 + b via custom op (1 DVE op)
                _, _, _, _, na1, nb = FORM[f]
                nc.vector.ln_bwd_dx(t(f + "_a")[:, :], z0, z1, col(na1),
                                    col(nb))

            def m2(f):      # tmp2 = a2*z2
                _, _, _, a2, _, _ = FORM[f]
                if ybatch and f.startswith("y"):
                    dst = yslice("ym2cat", f)
                else:
                    dst = t(f + "_m2")[:, :]
                ts_op(f + "m2", dst, z2, col(a2), 0.0)

            def ya1b(_=None):   # all three y a1 joins in one wide op
                mc = t("ymcat", 3 * FREE)[:, :].rearrange(
                    "p (o f) -> p o f", o=3)
                ac = t("yacat", 3 * FREE)[:, :].rearrange(
                    "p (o f) -> p o f", o=3)
                zb = z0.unsqueeze(1).broadcast_to([P, 3, FREE])
                nc.vector.tensor_add(out=ac, in0=zb, in1=mc)

            def ya2b(_=None):   # all three y a2 joins in one wide op
                nc.vector.tensor_add(out=t("ycat", 3 * FREE)[:, :],
                                     in0=t("yacat", 3 * FREE)[:, :],
                                     in1=t("ym2cat", 3 * FREE)[:, :])

            def a2(f):      # out = acc + tmp2
                E(f + "a2").tensor_add(out=t(f)[:, :], in0=t(f + "_a")[:, :],
                                       in1=t(f + "_m2")[:, :])

            def s2(f):      # out = acc + a2*z2 via stt (skip m2)
                _, _, _, a2c, _, _ = FORM[f]
                nc.vector.scalar_tensor_tensor(t(f)[:, :], z2, col(a2c),
                                               t(f + "_a")[:, :], OP.mult,
                                               OP.add)

            def e1m(_=None):   # e1 aux: tmp = a1*z2 + b
                ts_op("e1m1", t("e1_m1")[:, :], z2, col(C_A1E1),
                      col(C_BE1))

            def e1a(_=None):   # e1 = z1 + tmp
                nc.vector.tensor_add(out=t("e1")[:, :], in0=z1,
                                     in1=t("e1_m1")[:, :])

            def e1ln(_=None):
                nc.vector.ln_bwd_dx(t("e1")[:, :], z1, z2, col(C_NA1E1),
                                    col(C_NBE1))

            def sq(i):      # ACT squares: 0 <- e0, 1 <- e1, 2 <- z2
                src = {0: t("e0"), 1: t("e1")}.get(i)
                scol = {0: C_SQ0S, 1: C_SQ1S, 2: C_SQ2S}[i]
                dst = t(f"sq{i}")
                if i == 2:
                    nc.scalar.activation(dst[:, :], z2, AF.Square,
                                         bias=col(C_SQ2B), scale=col(scol))
                else:
                    nc.scalar.activation(dst[:, :], src[:, :], AF.Square,
                                         scale=col(scol))

            def d1(_=None):
                E("d1").tensor_add(out=t("d1")[:, :], in0=t("sq1")[:, :],
                                   in1=t("sq2")[:, :])

            def d2(_=None):
                E("d2").tensor_add(out=t("d2")[:, :], in0=t("d1")[:, :],
                                   in1=t("sq0")[:, :])

            def krbf(_=None):
                nc.scalar.activation(t("krbf")[:, :], t("d2")[:, :], AF.Exp,
                                     bias=col(C_BETA0), scale=col(C_NEGG))

            def p2(_=None):
                nc.scalar.activation(t("p2")[:, :], t("kl")[:, :], AF.Square,
                                     bias=col(C_P2B), scale=col(C_P2S))

            def tpoly(_=None):
                nc.vector.scalar_tensor_tensor(t("tpoly")[:, :],
                                               t("kl")[:, :], col(C_W1U0),
                                               t("p2")[:, :], OP.mult, op_w2)

            def tpm(_=None):
                ts_op("tpm", t("tp_m")[:, :], t("kl")[:, :], col(C_W1U0), 0.0)

            def tpa(_=None):
                if sw2_pos:
                    E("tpa").tensor_add(out=t("tpoly")[:, :],
                                        in0=t("tp_m")[:, :],
                                        in1=t("p2")[:, :])
                else:
                    E("tpa").tensor_sub(out=t("tpoly")[:, :],
                                        in0=t("tp_m")[:, :],
                                        in1=t("p2")[:, :])

            def t2(_=None):
                if polysq:
                    # poly+linear completed into p2; k = sA*(p2 +/- krbf)+k0
                    src = t("p2")
                    op_add = (sw0_pos == sw2_pos)
                else:
                    src = t("tpoly")
                    op_add = sw0_pos
                if op_add:
                    E("t2").tensor_add(out=t("t2")[:, :], in0=src[:, :],
                                       in1=t("krbf")[:, :])
                else:
                    E("t2").tensor_sub(out=t("t2")[:, :], in0=src[:, :],
                                       in1=t("krbf")[:, :])

            def th(_=None):
                if polysq:
                    nc.scalar.activation(t("th")[:, :], t("t2")[:, :],
                                         AF.Tanh, scale=col(C_THS),
                                         bias=col(C_THB))
                else:
                    nc.scalar.activation(t("th")[:, :], t("t2")[:, :],
                                         AF.Tanh, scale=0.5)

            def g1(o):
                if ybatch:
                    dst = yslice("gcat", f"y{o}")
                else:
                    dst = t(f"g1{o}")[:, :]
                if o in g1_act:
                    nc.scalar.activation(dst, t("th")[:, :],
                                         AF.Identity, bias=col(C_G1B + o),
                                         scale=col(C_G1S + o))
                else:
                    E(f"g1{o}").tensor_scalar(dst,
                                              t("th")[:, :], col(C_G1S + o),
                                              col(C_G1B + o), OP.mult,
                                              OP.add)

            def v(o, nsplit=1):
                vt = t(f"v{o}")
                cw = FREE // nsplit
                for sdx in range(nsplit):
                    sl = (slice(None), slice(sdx * cw, (sdx + 1) * cw))
                    if ybatch:
                        yin = t("ycat", 3 * FREE)[
                            :, o * FREE + sdx * cw:o * FREE + (sdx + 1) * cw]
                        gin = t("gcat", 3 * FREE)[
                            :, o * FREE + sdx * cw:o * FREE + (sdx + 1) * cw]
                    else:
                        yin = t(f"y{o}")[sl]
                        gin = t(f"g1{o}")[sl]
                    E(f"v{o}").tensor_mul(out=vt[sl], in0=yin, in1=gin)
                    fin = vt
                    if not bo_zero:
                        fin = t(f"f{o}")
                        nc.vector.tensor_scalar(fin[sl], vt[sl], 1.0,
                                                col(C_BO + o), OP.mult,
                                                OP.add)
                    dmaeng(out_eng[o]).dma_start(out=outs[o][sl],
                                                 in_=fin[sl])

            # ---------- emission schedule ----------
            # Pool-assigned mul ops are emitted when their step comes up;
            # engine in-order sequencing follows emission order per engine.
            steps = {
                "e1m": e1m, "e1a": e1a, "e1ln": e1ln, "d1": d1, "d2": d2,
                "krbf": krbf, "p2": p2, "tpoly": tpoly, "tpm": tpm,
                "tpa": tpa, "t2": t2, "th": th,
                "ya1b": ya1b, "ya2b": ya2b,
            }
            for f in FORM:
                steps[f + "m1"] = (lambda ff: lambda _=None: m1(ff))(f)
                steps[f + "a1"] = (lambda ff: lambda _=None: a1(ff))(f)
                steps[f + "ln1"] = (lambda ff: lambda _=None: ln1(ff))(f)
                steps[f + "m2"] = (lambda ff: lambda _=None: m2(ff))(f)
                steps[f + "a2"] = (lambda ff: lambda _=None: a2(ff))(f)
                steps[f + "s2"] = (lambda ff: lambda _=None: s2(ff))(f)
            for o in range(3):
                steps[f"sq{o}"] = (lambda oo: lambda _=None: sq(oo))(o)
                steps[f"g1{o}"] = (lambda oo: lambda _=None: g1(oo))(o)
                steps[f"v{o}"] = (lambda oo: lambda _=None: v(oo))(o)
                steps[f"v{o}s"] = (lambda oo: lambda _=None: v(oo, 2))(o)

            if dve_order is None:
                dve_order = DEFAULT_ORDER
            for s in dve_order:
                if polysq and s in ("tpm", "tpa", "tpoly"):
                    continue
                steps[s]()
    nc.compile()
    return nc


# Default schedule: e1 first (z2 arrives early on pool queue), kl path
# early (feeds p2 before krbf), e0 path, squares interleave on ACT, y
# mac work fills DVE while ACT runs, tail g/v lanes. Pool-assigned ops
# appear in the order too (per-engine in-order follows emission order).
DEFAULT_ORDER = (
    "klm1", "e0m1", "y0m1",           # dve ts of z1 (start asap)
    "sq2", "e1m", "y0m2",             # act: z2 square + offloaded affines
    "e0m2", "klm2", "y1m1", "y2m1", "y1m2", "y2m2",   # pool ts queue
    "kla1", "e1a", "e0a1",
    "sq1",
    "kla2", "e0a2",
    "sq0",
    "y0a1", "d1", "d2",
    "p2", "krbf",
    "y1a1", "tpoly", "t2",
    "th",
    "y0a2", "y1a2", "y2a1", "y2a2",
    "g10", "g11", "g12",
    "v0", "v1", "v2",
)


def _get_nc(sw0_pos, sw2_pos, bo_zero, cfg=None, polysq=True):
    def freeze(v):
        if isinstance(v, dict):
            return tuple(sorted(v.items()))
        return v
    key = (sw0_pos, sw2_pos, bo_zero, polysq,
           tuple(sorted((k, freeze(v)) for k, v in (cfg or {}).items())))
    if key not in _NC_CACHE:
        _NC_CACHE[key] = _build_nc(sw0_pos, sw2_pos, bo_zero, cfg, polysq)
    return _NC_CACHE[key]


def _host_prep(inputs):
    d = {k: np.asarray(v, dtype=np.float64) for k, v in inputs.items()}
    z = np.asarray(inputs["z"], dtype=np.float32)
    B, C, H, W = z.shape
    Wz, bz = d["z_proj_w"], d["z_proj_b"]
    Wt, bt = d["text_proj_w"], d["text_proj_b"]
    Wo, bo = d["out_w"], d["out_b"]
    gamma = np.exp(d["log_gamma"])
    alpha, c, w = d["alpha"], d["c"], d["w"]
    sumw = w.sum() + 1e-8
    w0p, w1p, w2p = w[0] / sumw, w[1] / sumw, w[2] / sumw

    t = d["text_vec"] @ Wt.T + bt                      # [B, HID]
    G = Wz.T @ Wz
    L = np.linalg.cholesky(G)                          # may raise
    delta = bz[None, :] - t
    v = delta @ Wz                                     # [B, 3]
    cdist = (delta ** 2).sum(1)
    r = np.linalg.solve(L, v.T).T                      # [B, 3]
    rho = cdist - (r ** 2).sum(1)
    u = t @ Wz                                         # [B, 3]
    s = t @ bz                                         # [B]
    M = Wo @ Wz                                        # [3, 3]
    m = Wo @ bz                                        # [3]

    u0 = u[:, 0]
    piv = [L[0, 0], L[1, 1]] + [M[o, 0] for o in range(3)]
    if min(abs(np.asarray(piv))) < 1e-7 or np.any(np.abs(u0) < 1e-7):
        raise np.linalg.LinAlgError("degenerate pivot")

    if w0p == 0.0:
        beta0 = np.full(B, -1e30)
    else:
        beta0 = -gamma * rho + np.log(abs(w0p))
    sw2 = np.sqrt(abs(w2p))

    cb = np.zeros((B, NCONST), dtype=np.float64)
    cb[:, C_A1E0] = L[1, 0] / L[0, 0]
    cb[:, C_BE0] = r[:, 0] / L[0, 0]
    cb[:, C_A2E0] = L[2, 0] / L[0, 0]
    cb[:, C_SQ0S] = L[0, 0]
    cb[:, C_A1E1] = L[2, 1] / L[1, 1]
    cb[:, C_BE1] = r[:, 1] / L[1, 1]
    cb[:, C_SQ1S] = L[1, 1]
    cb[:, C_SQ2S] = L[2, 2]
    cb[:, C_SQ2B] = r[:, 2]
    cb[:, C_NEGG] = -gamma
    cb[:, C_BETA0] = beta0
    cb[:, C_A1KL] = u[:, 1] / u0
    cb[:, C_BKL] = s / u0
    cb[:, C_A2KL] = u[:, 2] / u0
    # completed square: w1p*kl + w2p*(alpha*kl+c)^2 = sA*(s*kl' + b)^2 + k0
    # with kl = u0*kl'; falls back to the tpoly path when degenerate.
    polysq = bool(abs(w2p) * alpha ** 2 > 1e-12)
    sign_a = 1.0 if w2p >= 0 else -1.0
    if polysq:
        sabs = alpha * sw2 * np.abs(u0)              # sqrt|A| per batch
        bp = w1p * u0 + 2.0 * w2p * alpha * c * u0
        bsq = bp * sign_a / (2.0 * sabs)
        kconst = w2p * c ** 2 - sign_a * bsq ** 2
        cb[:, C_P2S] = sabs
        cb[:, C_P2B] = bsq
        cb[:, C_THS] = 0.5 * sign_a
        cb[:, C_THB] = 0.5 * kconst
    else:
        cb[:, C_P2S] = alpha * sw2 * u0
        cb[:, C_P2B] = c * sw2
    cb[:, C_W1U0] = w1p * u0
    for o in range(3):
        cb[:, C_A1Y + o] = M[o, 1] / M[o, 0]
        cb[:, C_BY + o] = m[o] / M[o, 0]
        cb[:, C_A2Y + o] = M[o, 2] / M[o, 0]
        cb[:, C_G1S + o] = 0.5 * M[o, 0]
        cb[:, C_G1B + o] = 1.5 * M[o, 0]
        cb[:, C_BO + o] = bo[o]
    # negated copies for ln_bwd (out = in0 - in1*s0 - s1)
    cb[:, C_NA1E0] = -cb[:, C_A1E0]
    cb[:, C_NBE0] = -cb[:, C_BE0]
    cb[:, C_NA1E1] = -cb[:, C_A1E1]
    cb[:, C_NBE1] = -cb[:, C_BE1]
    cb[:, C_NA1KL] = -cb[:, C_A1KL]
    cb[:, C_NBKL] = -cb[:, C_BKL]
    for o in range(3):
        cb[:, C_NA1Y + o] = -cb[:, C_A1Y + o]
        cb[:, C_NBY + o] = -cb[:, C_BY + o]
    cb = cb.astype(np.float32)

    z16 = z.astype(np.float16)
    in_maps = []
    for core in range(NCORES):
        cs = np.empty((P, NCONST), dtype=np.float32)
        z01a = np.empty((P, 2 * FREE), dtype=np.float16)
        z2a = np.empty((P, FREE), dtype=np.float16)
        for j in range(BPC):
            b = core * BPC + j
            pl = z16[b].reshape(3, ROWS, FREE)
            rs = slice(j * ROWS, (j + 1) * ROWS)
            z01a[rs, 0:FREE] = pl[1]
            z01a[rs, FREE:2 * FREE] = pl[0]
            z2a[rs, :] = pl[2]
            cs[rs, :] = cb[b]
        in_maps.append({"consts": cs, "z01": z01a, "z2": z2a})
    flags = (bool(w0p >= 0.0), bool(w2p >= 0.0),
             bool(np.all(bo == 0.0)), polysq)
    return in_maps, flags, (B, C, H, W)


def _numpy_fallback(inputs):
    d = {k: np.asarray(v, dtype=np.float64) for k, v in inputs.items()}
    z, Wz, bz = d["z"], d["z_proj_w"], d["z_proj_b"]
    t = d["text_vec"] @ d["text_proj_w"].T + d["text_proj_b"]
    zm = np.einsum("bchw,oc->bohw", z, Wz) + bz[None, :, None, None]
    gamma = np.exp(d["log_gamma"])
    diff = zm - t[:, :, None, None]
    dist = (diff * diff).sum(1)
    klin = np.einsum("bchw,bc->bhw", zm, t)
    krbf = np.exp(-gamma * dist)
    kpoly = (d["alpha"] * klin + d["c"]) ** 2
    w = d["w"]
    k = (w[0] * krbf + w[1] * klin + w[2] * kpoly) / (w.sum() + 1e-8)
    zf = zm * (1.0 + 1.0 / (1.0 + np.exp(-k[:, None])))
    out = np.einsum("bchw,oc->bohw", zf, d["out_w"]) + d["out_b"][None, :, None, None]
    return out.astype(np.float32)


def run(inputs, trace=False, cfg=None):
    if cfg is None:
        cfg = BEST_CFG
    try:
        in_maps, (sw0, sw2, boz, psq), (B, C, H, W) = _host_prep(inputs)
    except np.linalg.LinAlgError:
        return _numpy_fallback(inputs), None
    nc = _get_nc(sw0, sw2, boz, cfg, psq)
    res = bass_utils.run_bass_kernel_spmd(
        nc, in_maps, core_ids=list(range(NCORES)), trace=trace)
    out = np.empty((B, C, H, W), dtype=np.float32)
    for core in range(NCORES):
        r = res.results[core]
        for j in range(BPC):
            b = core * BPC + j
            rs = slice(j * ROWS, (j + 1) * ROWS)
            for o in range(3):
                out[b, o] = np.asarray(r[f"o{o}"][rs, :],
                                       dtype=np.float32).reshape(H, W)
    return out, res


def kernel(**inputs):
    out, _ = run(inputs, trace=False)
    return out
